# revision 1
# baseline (speedup 1.0000x reference)
"""Distributed GAT (3-layer, heads=1) Bass kernel for 8 TRN2 NeuronCores.

Strategy (dst-sharded, dst-major padded slots):
- Host: add self-loops, permute nodes by in-degree into degree-homogeneous
  blocks of 128, deal blocks round-robin to 8 cores, build per-core static
  gather-slot index arrays [128, S] (partition = dst node within block,
  column = padded in-edge rank; pads point at a sentinel table row).
- Device per layer:
    node pass:  r = h @ (W @ Q) and alpha_d = h @ (W @ adst) per 128-node
                block via one matmul (Q = Householder, Q e0 = asrc/|asrc|,
                so r[:,0]*|asrc| = alpha_s — no separate edge gather for it).
    exchange:   AllGather of the per-core table slice -> full [N,64] table.
    edge pass:  per block: gather in-edge rows (indirect DMA, one column of
                128 rows per instruction), e = Lrelu(r0*s + alpha_d) on ACT,
                w = Exp(e) with fused denominator accumulation, alpha = w/denom,
                weighted sum over the padded-degree axis on DVE,
                reconstruct h_next = (agg @ Q) + b (Q symmetric), relu.
  Pad slots gather a sentinel row of -1e38 -> w = exp(-inf) = 0.
- Pooling: segment-max over graphs via the same dst-major trick from a local
  node-major copy (sentinel -3e38), AllReduce(max) over cores, then fc +
  log_softmax computed redundantly on every core; core 0's output is returned.
"""
import sys

sys.path.insert(0, "/opt/trn_rl_repo")

import numpy as np

import concourse.bass as bass
import concourse.bacc as bacc
import concourse.tile as tile
import concourse.mybir as mybir
from concourse import bass_utils
from concourse.masks import make_identity

N_CORES = 8
D = 64
N_LAYERS = 3
N_CLASSES = 10
N_GRAPHS = 512
NEG_SLOPE = 0.2
P = 128

_COMPILED = {}


def _householder_first_col(a):
    """Orthogonal symmetric Q with Q @ e0 = a/||a||."""
    a = np.asarray(a, np.float64)
    ah = a / np.linalg.norm(a)
    e0 = np.zeros_like(ah)
    e0[0] = 1.0
    u = ah - e0
    nu = np.linalg.norm(u)
    if nu < 1e-12:
        return np.eye(len(a))
    u = u / nu
    return np.eye(len(a)) - 2.0 * np.outer(u, u)


def _host_prep(x, edge_index, batch, Ws, a_src, a_dst, bs, fc_w, fc_b):
    N = x.shape[0]
    E0 = edge_index.shape[1]
    src0 = np.asarray(edge_index[0], np.int64)
    dst0 = np.asarray(edge_index[1], np.int64)
    batch = np.asarray(batch, np.int64)

    NBLK_TOT = -(-N // P)
    NBLK_TOT = ((NBLK_TOT + N_CORES - 1) // N_CORES) * N_CORES  # multiple of 8
    NPAD = NBLK_TOT * P          # padded node count
    NB = NBLK_TOT // N_CORES     # blocks per core
    NPC = NB * P                 # nodes per core

    # in-degree with self-loops on all padded nodes
    deg = np.ones(NPAD, np.int64)
    np.add.at(deg, dst0, 1)

    # permutation: sort by degree desc (real nodes first automatically since
    # dummies have deg 1 which sorts low but may tie with real deg-1 nodes; fine)
    order = np.argsort(-deg, kind="stable")      # old id at each new position
    new_id = np.empty(NPAD, np.int64)
    new_id[order] = np.arange(NPAD)

    # block rank k -> core k % 8, slot j = k // 8
    # node new position pos -> block rank k = pos // P, p = pos % P
    # core-local numbering: local = j * P + p ; global table row = pos itself?
    # Table row order = concat over cores of their local nodes:
    #   row(c, j, p) = c * NPC + j * P + p.  Map pos -> row:
    k = np.arange(NPAD) // P
    p_in_blk = np.arange(NPAD) % P
    row_of_pos = (k % N_CORES) * NPC + (k // N_CORES) * P + p_in_blk
    # node old -> table row
    row_of_old = row_of_pos[new_id]

    SENT = NPAD  # sentinel table row

    # edges with self-loops, mapped to table rows
    src = np.concatenate([row_of_old[src0], row_of_old[np.arange(NPAD)]])
    dst = np.concatenate([row_of_old[dst0], row_of_old[np.arange(NPAD)]])

    # group in-edges by dst
    eorder = np.argsort(dst, kind="stable")
    src_s = src[eorder]
    dst_s = dst[eorder]
    degr = np.zeros(NPAD, np.int64)
    np.add.at(degr, dst, 1)
    starts = np.zeros(NPAD + 1, np.int64)
    np.cumsum(degr, out=starts[1:])

    # per (core, block-slot j) capacity C[j] = max deg in rank group, shared
    deg_by_row = degr  # indexed by table row
    deg_blocks = deg_by_row.reshape(NBLK_TOT, P)  # [core*NB + j, p] in row order
    Cmax = deg_blocks.max(axis=1).reshape(N_CORES, NB)
    C = Cmax.max(axis=0)  # [NB] shared across cores
    offs = np.zeros(NB + 1, np.int64)
    np.cumsum(C, out=offs[1:])
    S = int(offs[-1])

    # idx arrays [core][128, S]
    idxs = np.full((N_CORES, P, S), SENT, np.int32)
    for c in range(N_CORES):
        for j in range(NB):
            base_row = c * NPC + j * P
            for p in range(P):
                r = base_row + p
                d0, d1 = starts[r], starts[r + 1]
                idxs[c, p, offs[j]:offs[j] + (d1 - d0)] = src_s[d0:d1]

    # pooling: graph of each table row (real nodes), dummies -> graph -1
    graph_of_row = np.full(NPAD, -1, np.int64)
    graph_of_row[row_of_old[:N]] = batch
    GB = N_GRAPHS // P  # graph blocks (4)
    # per core: local rows grouped by graph
    LSENT = NPC  # local sentinel row in h3loc
    pool_lists = [[[[] for _ in range(P)] for _ in range(GB)] for _ in range(N_CORES)]
    for c in range(N_CORES):
        g_loc = graph_of_row[c * NPC:(c + 1) * NPC]
        for loc in range(NPC):
            g = g_loc[loc]
            if g >= 0:
                pool_lists[c][g // P][g % P].append(loc)
    PC = np.zeros(GB, np.int64)
    for q in range(GB):
        PC[q] = max(
            max((len(pool_lists[c][q][pp]) for pp in range(P)), default=0)
            for c in range(N_CORES)
        )
        PC[q] = max(PC[q], 1)
    poffs = np.zeros(GB + 1, np.int64)
    np.cumsum(PC, out=poffs[1:])
    SG = int(poffs[-1])
    pool_idx = np.full((N_CORES, P, SG), LSENT, np.int32)
    for c in range(N_CORES):
        for q in range(GB):
            for pp in range(P):
                lst = pool_lists[c][q][pp]
                pool_idx[c, pp, poffs[q]:poffs[q] + len(lst)] = lst

    # weights
    Ws = np.asarray(Ws, np.float64)
    a_src = np.asarray(a_src, np.float64)
    a_dst = np.asarray(a_dst, np.float64)
    bs = np.asarray(bs, np.float64)
    NR = np.zeros((N_LAYERS, D, D + 1), np.float64)
    Qs = np.zeros((N_LAYERS, D, D), np.float64)
    s_l = np.zeros(N_LAYERS)
    for l in range(N_LAYERS):
        Q = _householder_first_col(a_src[l])
        Qs[l] = Q
        s_l[l] = np.linalg.norm(a_src[l])
        NR[l, :, :D] = Ws[l] @ Q
        NR[l, :, D] = Ws[l] @ a_dst[l]

    # xT per core [64, NPC] in table-row order
    xpad = np.zeros((NPAD, D), np.float32)
    xpad[row_of_old[:N]] = np.asarray(x, np.float32)
    xT = np.stack([xpad[c * NPC:(c + 1) * NPC].T.copy() for c in range(N_CORES)])

    host = dict(
        NPAD=NPAD, NB=NB, NPC=NPC, S=S, SG=SG, GB=GB,
        C=C.astype(int), offs=offs.astype(int),
        PC=PC.astype(int), poffs=poffs.astype(int),
        s_l=s_l,
    )
    per_core = []
    for c in range(N_CORES):
        per_core.append({
            "xT": np.ascontiguousarray(xT[c]),
            "idxs": np.ascontiguousarray(idxs[c]),
            "pool_idx": np.ascontiguousarray(pool_idx[c]),
            "NR": np.ascontiguousarray(NR.transpose(1, 0, 2).reshape(D, N_LAYERS * (D + 1)).astype(np.float32)),
            "Qs": np.ascontiguousarray(Qs.transpose(1, 0, 2).reshape(D, N_LAYERS * D).astype(np.float32)),
            "bcol": np.ascontiguousarray(bs.T.astype(np.float32)),          # [64, 3]
            "brow3": np.ascontiguousarray(
                np.tile(bs[2][None, :].astype(np.float32), (P, 1))),        # [128, 64]
            "fcwT": np.ascontiguousarray(np.asarray(fc_w, np.float32).T),   # [64, 10]
            "fcb": np.ascontiguousarray(
                np.tile(np.asarray(fc_b, np.float32)[None, :], (P, 1))),    # [128, 10]
        })
    return host, per_core


def _build(host):
    NB, NPC, S, SG, GB = host["NB"], host["NPC"], host["S"], host["SG"], host["GB"]
    NPAD = host["NPAD"]
    C, offs = host["C"], host["offs"]
    PC, poffs = host["PC"], host["poffs"]
    s_l = host["s_l"]
    f32 = mybir.dt.float32
    AF = mybir.ActivationFunctionType
    OP = mybir.AluOpType

    nc = bacc.Bacc("TRN2", target_bir_lowering=False, debug=False, num_devices=N_CORES)
    t_xT = nc.dram_tensor("xT", [D, NPC], f32, kind="ExternalInput")
    t_idxs = nc.dram_tensor("idxs", [P, S], mybir.dt.int32, kind="ExternalInput")
    t_pidx = nc.dram_tensor("pool_idx", [P, SG], mybir.dt.int32, kind="ExternalInput")
    t_NR = nc.dram_tensor("NR", [D, N_LAYERS * (D + 1)], f32, kind="ExternalInput")
    t_Qs = nc.dram_tensor("Qs", [D, N_LAYERS * D], f32, kind="ExternalInput")
    t_bcol = nc.dram_tensor("bcol", [D, N_LAYERS], f32, kind="ExternalInput")
    t_brow3 = nc.dram_tensor("brow3", [P, D], f32, kind="ExternalInput")
    t_fcwT = nc.dram_tensor("fcwT", [D, N_CLASSES], f32, kind="ExternalInput")
    t_fcb = nc.dram_tensor("fcb", [P, N_CLASSES], f32, kind="ExternalInput")
    t_out = nc.dram_tensor("out", [N_GRAPHS, N_CLASSES], f32, kind="ExternalOutput")

    with tile.TileContext(nc) as tc:
        with (
            tc.tile_pool(name="persist", bufs=1) as pp,
            tc.tile_pool(name="work", bufs=6) as wp,
            tc.tile_pool(name="gat", bufs=6) as gp,
            tc.tile_pool(name="psum", bufs=1, space="PSUM") as ps,
            tc.tile_pool(name="dram", bufs=1, space="DRAM") as dp,
        ):
            # persistent SBUF
            hT = pp.tile([D, NPC], f32)
            idxs = pp.tile([P, S], mybir.dt.int32)
            pidx = pp.tile([P, SG], mybir.dt.int32)
            NRt = pp.tile([D, N_LAYERS * (D + 1)], f32)
            Qst = pp.tile([D, N_LAYERS * D], f32)
            bcol = pp.tile([D, N_LAYERS], f32)
            brow3 = pp.tile([P, D], f32)
            fcwT = pp.tile([D, N_CLASSES], f32)
            fcb = pp.tile([P, N_CLASSES], f32)
            ad = pp.tile([P, NB], f32)
            ident = pp.tile([P, P], f32)
            sentT = pp.tile([1, D], f32)
            sentP = pp.tile([1, D], f32)
            nc.sync.dma_start(hT[:], t_xT[:])
            nc.sync.dma_start(idxs[:], t_idxs[:])
            nc.sync.dma_start(pidx[:], t_pidx[:])
            nc.sync.dma_start(NRt[:], t_NR[:])
            nc.sync.dma_start(Qst[:], t_Qs[:])
            nc.sync.dma_start(bcol[:], t_bcol[:])
            nc.sync.dma_start(brow3[:], t_brow3[:])
            nc.sync.dma_start(fcwT[:], t_fcwT[:])
            nc.sync.dma_start(fcb[:], t_fcb[:])
            make_identity(nc, ident[:])
            nc.vector.memset(sentT[:], -1.0e38)
            nc.vector.memset(sentP[:], -3.0e38)

            # DRAM
            tabA = dp.tile([NPC, D], f32)                  # local node-pass out
            tabFull = dp.tile([NPAD + 1, D], f32)          # allgathered + sentinel
            h3loc = dp.tile([NPC + 1, D], f32)             # final h, node-major
            gpart = dp.tile([N_GRAPHS, D], f32)            # local graph max
            gall = dp.tile([N_GRAPHS, D], f32)             # reduced graph max

            for l in range(N_LAYERS):
                # ---- node pass ----
                for j in range(NB):
                    np_ps = ps.tile([P, D + 1], f32, tag="npp")
                    nc.tensor.matmul(
                        out=np_ps[:],
                        lhsT=hT[:, j * P:(j + 1) * P],
                        rhs=NRt[:, l * (D + 1):(l + 1) * (D + 1)],
                        start=True, stop=True,
                    )
                    rows = wp.tile([P, D], f32, tag="rows")
                    nc.vector.tensor_copy(out=rows[:], in_=np_ps[:, :D])
                    nc.scalar.copy(out=ad[:, j:j + 1], in_=np_ps[:, D:D + 1])
                    nc.sync.dma_start(tabA[j * P:(j + 1) * P, :], rows[:])
                # ---- exchange ----
                nc.gpsimd.collective_compute(
                    "AllGather", mybir.AluOpType.bypass,
                    replica_groups=[list(range(N_CORES))],
                    ins=[tabA[:].opt()],
                    outs=[tabFull[0:NPAD, :].opt()],
                )
                nc.sync.dma_start(tabFull[NPAD:NPAD + 1, :], sentT[:])
                # ---- edge pass ----
                for j in range(NB):
                    Cj = int(C[j])
                    if Cj == 0:
                        continue
                    g = gp.tile([P, Cj * D], f32, tag="g")
                    for cc in range(Cj):
                        nc.gpsimd.indirect_dma_start(
                            out=g[:, cc * D:(cc + 1) * D],
                            out_offset=None,
                            in_=tabFull[:, :],
                            in_offset=bass.IndirectOffsetOnAxis(
                                ap=idxs[:, offs[j] + cc:offs[j] + cc + 1], axis=0),
                        )
                    g3 = g[:].rearrange("p (c d) -> p c d", d=D)
                    r0 = g3[:, :, 0:1].rearrange("p c one -> p (c one)")
                    ew = wp.tile([P, Cj], f32, tag="ew")
                    nc.scalar.activation(
                        out=ew[:], in_=r0, func=AF.Lrelu,
                        bias=ad[:, j:j + 1], scale=float(s_l[l]), alpha=NEG_SLOPE)
                    w = wp.tile([P, Cj], f32, tag="w")
                    denom = wp.tile([P, 1], f32, tag="denom")
                    nc.scalar.activation(out=w[:], in_=ew[:], func=AF.Exp,
                                         accum_out=denom[:])
                    recip = wp.tile([P, 1], f32, tag="recip")
                    nc.vector.reciprocal(out=recip[:], in_=denom[:])
                    alpha = wp.tile([P, Cj], f32, tag="alpha")
                    nc.vector.tensor_scalar_mul(out=alpha[:], in0=w[:], scalar1=recip[:])
                    nc.vector.tensor_tensor(
                        out=g3, in0=g3,
                        in1=alpha[:].to_broadcast([P, Cj, D]),
                        op=OP.mult)
                    agg = wp.tile([P, D], f32, tag="agg")
                    nc.vector.reduce_sum(
                        out=agg[:], in_=g3.rearrange("p c d -> p d c"),
                        axis=mybir.AxisListType.X)
                    aggT_ps = ps.tile([D, P], f32, tag="aggT")
                    nc.tensor.transpose(out=aggT_ps[:], in_=agg[:], identity=ident[:])
                    aggT = wp.tile([D, P], f32, tag="aggTs")
                    nc.vector.tensor_copy(out=aggT[:], in_=aggT_ps[:])
                    if l < N_LAYERS - 1:
                        h_ps = ps.tile([D, P], f32, tag="hps")
                        nc.tensor.matmul(
                            out=h_ps[:], lhsT=Qst[:, l * D:(l + 1) * D], rhs=aggT[:],
                            start=True, stop=True)
                        nc.scalar.activation(
                            out=hT[:, j * P:(j + 1) * P], in_=h_ps[:],
                            func=AF.Relu, bias=bcol[:, l:l + 1])
                    else:
                        h3_ps = ps.tile([P, D], f32, tag="h3ps")
                        nc.tensor.matmul(
                            out=h3_ps[:], lhsT=aggT[:], rhs=Qst[:, l * D:(l + 1) * D],
                            start=True, stop=True)
                        h3 = wp.tile([P, D], f32, tag="h3")
                        nc.vector.tensor_tensor(
                            out=h3[:], in0=h3_ps[:],
                            in1=brow3[:], op=OP.add)
                        nc.sync.dma_start(h3loc[j * P:(j + 1) * P, :], h3[:])
            # ---- pooling ----
            nc.sync.dma_start(h3loc[NPC:NPC + 1, :], sentP[:])
            for q in range(GB):
                PCq = int(PC[q])
                pg = gp.tile([P, PCq * D], f32, tag="pg")
                for cc in range(PCq):
                    nc.gpsimd.indirect_dma_start(
                        out=pg[:, cc * D:(cc + 1) * D],
                        out_offset=None,
                        in_=h3loc[:, :],
                        in_offset=bass.IndirectOffsetOnAxis(
                            ap=pidx[:, poffs[q] + cc:poffs[q] + cc + 1], axis=0),
                    )
                pg3 = pg[:].rearrange("p (c d) -> p c d", d=D)
                gmax = wp.tile([P, D], f32, tag="gmax")
                nc.vector.reduce_max(
                    out=gmax[:], in_=pg3.rearrange("p c d -> p d c"),
                    axis=mybir.AxisListType.X)
                nc.sync.dma_start(gpart[q * P:(q + 1) * P, :], gmax[:])
            nc.gpsimd.collective_compute(
                "AllReduce", mybir.AluOpType.max,
                replica_groups=[list(range(N_CORES))],
                ins=[gpart[:].opt()],
                outs=[gall[:].opt()],
            )
            # ---- fc + log_softmax (redundant on all cores) ----
            for q in range(GB):
                gsb = wp.tile([P, D], f32, tag="gsb")
                nc.sync.dma_start(gsb[:], gall[q * P:(q + 1) * P, :])
                mask = wp.tile([P, D], f32, tag="mask")
                nc.vector.tensor_scalar(
                    out=mask[:], in0=gsb[:], scalar1=-1.0e37, scalar2=None,
                    op0=OP.is_gt)
                nc.vector.tensor_tensor(out=gsb[:], in0=gsb[:], in1=mask[:],
                                        op=OP.mult)
                gT_ps = ps.tile([D, P], f32, tag="gT")
                nc.tensor.transpose(out=gT_ps[:], in_=gsb[:], identity=ident[:])
                gT = wp.tile([D, P], f32, tag="gTs")
                nc.vector.tensor_copy(out=gT[:], in_=gT_ps[:])
                lg_ps = ps.tile([P, N_CLASSES], f32, tag="lg")
                nc.tensor.matmul(out=lg_ps[:], lhsT=gT[:], rhs=fcwT[:],
                                 start=True, stop=True)
                lg = wp.tile([P, N_CLASSES], f32, tag="lgs")
                nc.vector.tensor_tensor(
                    out=lg[:], in0=lg_ps[:],
                    in1=fcb[:], op=OP.add)
                m = wp.tile([P, 1], f32, tag="m")
                nc.vector.reduce_max(out=m[:], in_=lg[:], axis=mybir.AxisListType.X)
                mneg = wp.tile([P, 1], f32, tag="mneg")
                nc.vector.tensor_scalar_mul(out=mneg[:], in0=m[:], scalar1=-1.0)
                ex = wp.tile([P, N_CLASSES], f32, tag="ex")
                sumex = wp.tile([P, 1], f32, tag="sumex")
                nc.scalar.activation(out=ex[:], in_=lg[:], func=AF.Exp,
                                     bias=mneg[:], accum_out=sumex[:])
                logz = wp.tile([P, 1], f32, tag="logz")
                nc.scalar.activation(out=logz[:], in_=sumex[:], func=AF.Ln)
                off = wp.tile([P, 1], f32, tag="off")
                nc.vector.tensor_add(out=off[:], in0=m[:], in1=logz[:])
                outsb = wp.tile([P, N_CLASSES], f32, tag="outsb")
                nc.vector.tensor_tensor(
                    out=outsb[:], in0=lg[:],
                    in1=off[:].to_broadcast([P, N_CLASSES]), op=OP.subtract)
                nc.sync.dma_start(t_out[q * P:(q + 1) * P, :], outsb[:])
    nc.compile()
    return nc


def kernel(**inputs):
    x = np.asarray(inputs["x"])
    key = (x.shape, inputs["edge_index"].shape)
    host, per_core = _host_prep(**inputs)
    if key not in _COMPILED:
        _COMPILED[key] = _build(host)
    nc = _COMPILED[key]
    in_maps = [per_core[c] for c in range(N_CORES)]
    import os
    trace = False
    if os.environ.get("KERNEL_TRACE") == "1":
        try:
            import types
            if "antenv.axon_hooks" not in sys.modules:
                import antenv
                from trn_agent_boot.trn_boot import _ntff_profile_via_ctypes
                mod = types.ModuleType("antenv.axon_hooks")
                _state = {"hook": _ntff_profile_via_ctypes("/opt/axon/libaxon_pjrt.so")}
                mod.set_axon_ntff_profile_hook = lambda h: _state.__setitem__("hook", h)
                mod.get_axon_ntff_profile_hook = lambda: _state["hook"]
                sys.modules["antenv.axon_hooks"] = mod
                antenv.axon_hooks = mod
            trace = True
        except Exception:
            trace = False
    res = bass_utils.run_bass_kernel_spmd(
        nc, in_maps, core_ids=list(range(N_CORES)), trace=trace)
    globals()['LAST_EXEC_NS'] = res.exec_time_ns
    return np.asarray(res.results[0]["out"], np.float32)


LAST_EXEC_NS = None



# revision 7
# speedup vs baseline: 1.0652x; 1.0652x over previous
"""Distributed GAT (3-layer, heads=1) Bass kernel for 8 TRN2 NeuronCores.

Strategy (dst-sharded, batched dma_gather over a bf16 pair-row table):
- Host: permute nodes by in-degree (excl. self-loop) into degree-homogeneous
  blocks of 128, deal blocks round-robin to 8 cores. Table row r = node;
  pair-row i = nodes (2i, 2i+1) packed as 128 bf16 = 256 B, so pair indices
  fit int16 (max 25087 < 32767) and one InstDMAGatherAnt fetches thousands
  of rows per instruction (vs one 128-row indirect DMA per slot column).
- Blocks are grouped into chunks of G=4; per-chunk slot capacity Cc = max
  in-degree in the chunk's rank groups. Slots gather the PAIR containing the
  src node; a static half-mask kills the wrong half and pad slots.
- Device per layer:
    node pass:  per block one matmul r=[h@(W Q) | h@(W a_dst)] -> radj (bf16)
                kept in SBUF (self-loop contributions read locally) and
                DMA'd row-major to tabA.
    exchange:   AllGather (bf16) -> Shared tabFull [NPAD, 64].
    edge pass:  per chunk: dma_gather pair rows -> [128, G*Cc, 128] bf16;
                w = max(exp(e), exp(0.2 e)) (Exp-only scalar table; no Lrelu
                table thrash), masked; unnormalized weighted sums via
                contiguous pairwise tree reduction (no strided reduce);
                add self term, normalize by the accumulated denominator,
                transpose+reconstruct through Q per 4-block PE group with
                fused bias+relu on DVE.
- Pooling: segment-max via dma_gather from local h3loc (sentinel -3e38),
  pairwise max tree, AllReduce(max), then fc + log_softmax on every core.
"""
import sys

sys.path.insert(0, "/opt/trn_rl_repo")

import numpy as np

import concourse.bass as bass
import concourse.bacc as bacc
import concourse.tile as tile
import concourse.mybir as mybir
from concourse import bass_utils
from concourse.masks import make_identity

N_CORES = 8
D = 64
N_LAYERS = 3
N_CLASSES = 10
N_GRAPHS = 512
NEG_SLOPE = 0.2
P = 128
GBLK = 4                  # blocks per chunk
MAX_GATHER_COLS = 118     # 128*cols idx ints must fit Q7 scratch (<=~16200)

_COMPILED = {}


def _householder_first_col(a):
    """Orthogonal symmetric Q with Q @ e0 = a/||a||."""
    a = np.asarray(a, np.float64)
    ah = a / np.linalg.norm(a)
    e0 = np.zeros_like(ah)
    e0[0] = 1.0
    u = ah - e0
    nu = np.linalg.norm(u)
    if nu < 1e-12:
        return np.eye(len(a))
    u = u / nu
    return np.eye(len(a)) - 2.0 * np.outer(u, u)


def _idx16_of(V):
    """[128, S] int -> int16 idx tile [128, 8S] (16-part pattern replicated x8).

    Gather position i = col*128 + p reads idx[i%16, i//16]; with
    i//16 = col*8 + p//16 the transform is a reshape/transpose.
    """
    Pn, S = V.shape
    assert Pn == 128
    t = V.reshape(8, 16, S).transpose(1, 2, 0).reshape(16, 8 * S)
    return np.tile(t.astype(np.int16), (8, 1))


def _host_prep(x, edge_index, batch, Ws, a_src, a_dst, bs, fc_w, fc_b):
    N = x.shape[0]
    src0 = np.asarray(edge_index[0], np.int64)
    dst0 = np.asarray(edge_index[1], np.int64)
    batch = np.asarray(batch, np.int64)

    NBLK_TOT = -(-N // P)
    NBLK_TOT = ((NBLK_TOT + N_CORES - 1) // N_CORES) * N_CORES
    NPAD = NBLK_TOT * P
    NB = NBLK_TOT // N_CORES
    NPC = NB * P
    NPAIR = NPAD // 2

    # in-degree (excluding self loops; those are handled locally on-chip)
    deg = np.zeros(NPAD, np.int64)
    np.add.at(deg, dst0, 1)
    order = np.argsort(-deg, kind="stable")
    new_id = np.empty(NPAD, np.int64)
    new_id[order] = np.arange(NPAD)
    k = np.arange(NPAD) // P
    p_in_blk = np.arange(NPAD) % P
    row_of_pos = (k % N_CORES) * NPC + (k // N_CORES) * P + p_in_blk
    row_of_old = row_of_pos[new_id]

    src_r = row_of_old[src0]
    dst_r = row_of_old[dst0]

    # per (core, block, partition) in-edge counts -> rank-group capacities
    core_of = dst_r // NPC
    j_of = (dst_r % NPC) // P
    p_of = dst_r % P
    cnt = np.zeros((N_CORES, NB, P), np.int64)
    np.add.at(cnt, (core_of, j_of, p_of), 1)
    C = cnt.max(axis=(0, 2))  # [NB] shared across cores

    # chunks of up to GBLK consecutive blocks, padded to chunk max
    chunks = []  # (j0, g, Cc, ocol)
    ocol = 0
    j0 = 0
    while j0 < NB:
        g = min(GBLK, NB - j0)
        Cc = int(C[j0:j0 + g].max())
        Cc = max(Cc, 1)
        chunks.append((j0, g, Cc, ocol))
        ocol += g * Cc
        j0 += g
    S = ocol

    # slot fill (vectorized): rank of each edge within its dst
    eorder = np.argsort(dst_r, kind="stable")
    dst_s = dst_r[eorder]
    src_s = src_r[eorder]
    dcount = np.zeros(NPAD + 1, np.int64)
    np.add.at(dcount, dst_s, 1)
    starts = np.zeros(NPAD + 1, np.int64)
    np.cumsum(dcount[:-1], out=starts[1:])
    rank_e = np.arange(len(dst_s)) - starts[dst_s]

    ci_of_j = np.zeros(NB, np.int64)
    colbase_of_j = np.zeros(NB, np.int64)
    Cc_of_j = np.zeros(NB, np.int64)
    for ci, (j0, g, Cc, oc) in enumerate(chunks):
        for gg in range(g):
            ci_of_j[j0 + gg] = ci
            colbase_of_j[j0 + gg] = oc + gg * Cc
            Cc_of_j[j0 + gg] = Cc

    ecore = dst_s // NPC
    ej = (dst_s % NPC) // P
    ep = dst_s % P
    ecol = colbase_of_j[ej] + rank_e

    slotpair = np.zeros((N_CORES, P, S), np.int64)
    halfmask = np.zeros((N_CORES, P, 2 * S), np.float32)
    slotpair[ecore, ep, ecol] = src_s // 2
    halfmask[ecore, ep, 2 * ecol + (src_s % 2)] = 1.0

    # pooling: local rows grouped by graph (graph block q = graphs 128q..)
    GB = N_GRAPHS // P
    graph_of_row = np.full(NPAD, -1, np.int64)
    graph_of_row[row_of_old[:N]] = batch
    pool_cnt = np.zeros((N_CORES, GB, P), np.int64)
    rr = np.arange(NPAD)
    real = graph_of_row >= 0
    prc = rr[real] // NPC
    pg = graph_of_row[real]
    np.add.at(pool_cnt, (prc, pg // P, pg % P), 1)
    PC = np.maximum(pool_cnt.max(axis=(0, 2)), 1)  # [GB]
    poffs = np.zeros(GB + 1, np.int64)
    np.cumsum(PC, out=poffs[1:])
    SG = int(poffs[-1])
    LSENT = NPC
    pool_slot = np.full((N_CORES, P, SG), LSENT, np.int64)
    # rank of local row within its (core, graph)
    lr = rr[real]
    lcore = prc
    lloc = lr % NPC
    pkey = lcore * N_GRAPHS + pg
    porder = np.argsort(pkey, kind="stable")
    pk_s = pkey[porder]
    lloc_s = lloc[porder]
    pstart = np.zeros(N_CORES * N_GRAPHS + 1, np.int64)
    pc2 = np.zeros(N_CORES * N_GRAPHS + 1, np.int64)
    np.add.at(pc2, pk_s, 1)
    np.cumsum(pc2[:-1], out=pstart[1:])
    prank = np.arange(len(pk_s)) - pstart[pk_s]
    pcore_s = pk_s // N_GRAPHS
    pgr_s = pk_s % N_GRAPHS
    pool_slot[pcore_s, pgr_s % P, poffs[pgr_s // P] + prank] = lloc_s

    # weights
    Ws = np.asarray(Ws, np.float64)
    a_src = np.asarray(a_src, np.float64)
    a_dst = np.asarray(a_dst, np.float64)
    bs = np.asarray(bs, np.float64)
    NR = np.zeros((N_LAYERS, D, D + 1), np.float64)
    Qs = np.zeros((N_LAYERS, D, D), np.float64)
    s_l = np.zeros(N_LAYERS)
    for l in range(N_LAYERS):
        Q = _householder_first_col(a_src[l])
        Qs[l] = Q
        s_l[l] = np.linalg.norm(a_src[l])
        NR[l, :, :D] = Ws[l] @ Q
        NR[l, :, D] = Ws[l] @ a_dst[l]

    xpad = np.zeros((NPAD, D), np.float32)
    xpad[row_of_old[:N]] = np.asarray(x, np.float32)

    import ml_dtypes
    bf = ml_dtypes.bfloat16

    host = dict(
        NPAD=NPAD, NB=NB, NPC=NPC, NPAIR=NPAIR, S=S, SG=SG, GB=GB,
        chunks=chunks, PC=PC.astype(int), poffs=poffs.astype(int),
        s_l=s_l,
    )
    per_core = []
    for c in range(N_CORES):
        per_core.append({
            "xT": np.ascontiguousarray(
                xpad[c * NPC:(c + 1) * NPC].T.astype(bf)),
            "idx": np.ascontiguousarray(_idx16_of(slotpair[c])),
            "pidx": np.ascontiguousarray(_idx16_of(pool_slot[c])),
            "hmask": np.ascontiguousarray(halfmask[c].astype(bf)),
            "NR": np.ascontiguousarray(
                NR.transpose(1, 0, 2).reshape(D, N_LAYERS * (D + 1)).astype(bf)),
            "Qs": np.ascontiguousarray(
                Qs.transpose(1, 0, 2).reshape(D, N_LAYERS * D).astype(np.float32)),
            "bcol": np.ascontiguousarray(bs.T.astype(np.float32)),
            "brow3": np.ascontiguousarray(
                np.tile(bs[2][None, :].astype(np.float32), (P, 1))),
            "fcwT": np.ascontiguousarray(np.asarray(fc_w, np.float32).T),
            "fcb": np.ascontiguousarray(
                np.tile(np.asarray(fc_b, np.float32)[None, :], (P, 1))),
        })
    return host, per_core


def _build(host):
    NB, NPC, S, SG, GB = host["NB"], host["NPC"], host["S"], host["SG"], host["GB"]
    NPAD, NPAIR = host["NPAD"], host["NPAIR"]
    chunks = host["chunks"]
    PC, poffs = host["PC"], host["poffs"]
    s_l = host["s_l"]
    f32 = mybir.dt.float32
    bf16 = mybir.dt.bfloat16
    i16 = mybir.dt.int16
    AF = mybir.ActivationFunctionType
    OP = mybir.AluOpType

    nc = bacc.Bacc("TRN2", target_bir_lowering=False, debug=False, num_devices=N_CORES)
    t_xT = nc.dram_tensor("xT", [D, NPC], bf16, kind="ExternalInput")
    t_idx = nc.dram_tensor("idx", [P, 8 * S], i16, kind="ExternalInput")
    t_pidx = nc.dram_tensor("pidx", [P, 8 * SG], i16, kind="ExternalInput")
    t_hmask = nc.dram_tensor("hmask", [P, 2 * S], bf16, kind="ExternalInput")
    t_NR = nc.dram_tensor("NR", [D, N_LAYERS * (D + 1)], bf16, kind="ExternalInput")
    t_Qs = nc.dram_tensor("Qs", [D, N_LAYERS * D], f32, kind="ExternalInput")
    t_bcol = nc.dram_tensor("bcol", [D, N_LAYERS], f32, kind="ExternalInput")
    t_brow3 = nc.dram_tensor("brow3", [P, D], f32, kind="ExternalInput")
    t_fcwT = nc.dram_tensor("fcwT", [D, N_CLASSES], f32, kind="ExternalInput")
    t_fcb = nc.dram_tensor("fcb", [P, N_CLASSES], f32, kind="ExternalInput")
    t_out = nc.dram_tensor("out", [N_GRAPHS, N_CLASSES], f32, kind="ExternalOutput")

    t_tabA = nc.dram_tensor("tabA", [NPC, D], bf16)
    t_tabFull = nc.dram_tensor("tabFull", [NPAD, D], bf16, addr_space="Shared")
    t_h3loc = nc.dram_tensor("h3loc", [NPC + 1, D], f32)
    t_gpart = nc.dram_tensor("gpart", [N_GRAPHS, D], f32)
    t_gall = nc.dram_tensor("gall", [N_GRAPHS, D], f32)

    with tile.TileContext(nc) as tc:
        with (
            tc.tile_pool(name="persist", bufs=1) as pp,
            tc.tile_pool(name="gt", bufs=2) as gp,
            tc.tile_pool(name="tree", bufs=1) as tp,
            tc.tile_pool(name="work", bufs=2) as wp,
            tc.tile_pool(name="psA", bufs=2, space="PSUM") as psA,
            tc.tile_pool(name="psB", bufs=1, space="PSUM") as psB,
        ):
            # persistent SBUF
            hT = pp.tile([D, NPC], bf16)
            radj = pp.tile([P, NB * (D + 1)], bf16)
            idxs = pp.tile([P, 8 * S], i16)
            pidx = pp.tile([P, 8 * SG], i16)
            hmask = pp.tile([P, 2 * S], bf16)
            NRt = pp.tile([D, N_LAYERS * (D + 1)], bf16)
            Qst = pp.tile([D, N_LAYERS * D], f32)
            bcol = pp.tile([D, N_LAYERS], f32)
            brow3 = pp.tile([P, D], f32)
            fcwT = pp.tile([D, N_CLASSES], f32)
            fcb = pp.tile([P, N_CLASSES], f32)
            ident = pp.tile([P, P], f32)
            sentP = pp.tile([1, D], f32)
            nc.sync.dma_start(hT[:], t_xT[:])
            nc.sync.dma_start(idxs[:], t_idx[:])
            nc.sync.dma_start(pidx[:], t_pidx[:])
            nc.sync.dma_start(hmask[:], t_hmask[:])
            nc.sync.dma_start(NRt[:], t_NR[:])
            nc.sync.dma_start(Qst[:], t_Qs[:])
            nc.sync.dma_start(bcol[:], t_bcol[:])
            nc.sync.dma_start(brow3[:], t_brow3[:])
            nc.sync.dma_start(fcwT[:], t_fcwT[:])
            nc.sync.dma_start(fcb[:], t_fcb[:])
            make_identity(nc, ident[:])
            nc.vector.memset(sentP[:], -3.0e38)
            nc.sync.dma_start(t_h3loc[NPC:NPC + 1, :], sentP[:])

            tab_pairs = t_tabFull[:].rearrange("(a b) d -> a (b d)", b=2)

            for l in range(N_LAYERS):
                sl = float(s_l[l])
                # ---- node pass ----
                for j in range(NB):
                    np_ps = psA.tile([P, D + 1], f32, tag="npp")
                    nc.tensor.matmul(
                        out=np_ps[:],
                        lhsT=hT[:, j * P:(j + 1) * P],
                        rhs=NRt[:, l * (D + 1):(l + 1) * (D + 1)],
                        start=True, stop=True,
                    )
                    nc.vector.tensor_copy(
                        out=radj[:, j * (D + 1):(j + 1) * (D + 1)], in_=np_ps[:])
                    nc.sync.dma_start(
                        t_tabA[j * P:(j + 1) * P, :],
                        radj[:, j * (D + 1):j * (D + 1) + D])
                # ---- exchange ----
                nc.gpsimd.collective_compute(
                    "AllGather", mybir.AluOpType.bypass,
                    replica_groups=[list(range(N_CORES))],
                    ins=[t_tabA[:].opt()],
                    outs=[t_tabFull[:].opt()],
                )
                # ---- self-loop weights for all blocks: w = max(e^e2, e^.2e2)
                r0s = radj[:].rearrange("p (j e) -> p j e", e=D + 1)[:, :, 0:1] \
                    .rearrange("p j one -> p (j one)")
                ads = radj[:].rearrange("p (j e) -> p j e", e=D + 1)[:, :, D:D + 1] \
                    .rearrange("p j one -> p (j one)")
                e2s = wp.tile([P, NB], f32, tag="e2s")
                nc.vector.tensor_scalar(
                    out=e2s[:], in0=r0s, scalar1=sl, scalar2=None, op0=OP.mult)
                nc.vector.tensor_tensor(out=e2s[:], in0=e2s[:], in1=ads, op=OP.add)
                wsA = wp.tile([P, NB], f32, tag="wsA")
                wself = wp.tile([P, NB], f32, tag="wself")
                nc.scalar.activation(out=wsA[:], in_=e2s[:], func=AF.Exp)
                nc.scalar.activation(out=wself[:], in_=e2s[:], func=AF.Exp,
                                     scale=NEG_SLOPE)
                nc.vector.tensor_tensor(out=wself[:], in0=wself[:], in1=wsA[:],
                                        op=OP.max)
                # ---- edge pass per chunk ----
                for (j0, g, Cc, oc) in chunks:
                    ncols = g * Cc
                    gt = gp.tile([P, ncols * P], bf16, tag="gt")
                    gt_g = gt[:].rearrange("p (k e) -> p k e", e=P)
                    a = 0
                    while a < ncols:
                        b = min(a + MAX_GATHER_COLS, ncols)
                        nidx = (b - a) * P
                        nc.gpsimd.dma_gather(
                            out_ap=gt_g[:, a:b, :],
                            in_ap=tab_pairs,
                            idxs_ap=idxs[:, 8 * (oc + a):8 * (oc + b)],
                            num_idxs=nidx,
                            num_idxs_reg=nidx,
                            elem_size=P,
                            single_packet=False,
                        )
                        a = b
                    # attention weights over [P, 2*ncols]
                    r0 = gt[:].rearrange("p (k e) -> p k e", e=D)[:, :, 0:1] \
                        .rearrange("p k one -> p (k one)")
                    ad_bc = radj[:].rearrange("p (j e) -> p j e", e=D + 1)[
                        :, j0:j0 + g, D:D + 1] \
                        .rearrange("p j one -> p (j one)") \
                        .to_broadcast([P, g, 2 * Cc])
                    e2 = wp.tile([P, 2 * ncols], f32, tag="e2")
                    nc.vector.tensor_scalar(
                        out=e2[:], in0=r0, scalar1=sl, scalar2=None, op0=OP.mult)
                    nc.vector.tensor_tensor(
                        out=e2[:].rearrange("p (g c) -> p g c", g=g),
                        in0=e2[:].rearrange("p (g c) -> p g c", g=g),
                        in1=ad_bc, op=OP.add)
                    wA = wp.tile([P, 2 * ncols], f32, tag="wA")
                    wm = wp.tile([P, 2 * ncols], f32, tag="wm")
                    nc.scalar.activation(out=wA[:], in_=e2[:], func=AF.Exp)
                    nc.scalar.activation(out=wm[:], in_=e2[:], func=AF.Exp,
                                         scale=NEG_SLOPE)
                    nc.vector.tensor_tensor(out=wm[:], in0=wm[:], in1=wA[:],
                                            op=OP.max)
                    nc.vector.tensor_tensor(
                        out=wm[:], in0=wm[:],
                        in1=hmask[:, 2 * oc:2 * (oc + ncols)], op=OP.mult)
                    # denominator per block
                    dn = wp.tile([P, g], f32, tag="dn")
                    nc.vector.reduce_sum(
                        out=dn[:],
                        in_=wm[:].rearrange("p (g c) -> p g c", g=g),
                        axis=mybir.AxisListType.X)
                    nc.vector.tensor_tensor(
                        out=dn[:], in0=dn[:], in1=wself[:, j0:j0 + g], op=OP.add)
                    recip = wp.tile([P, g], f32, tag="recip")
                    nc.vector.reciprocal(out=recip[:], in_=dn[:])
                    # weight the gathered rows (in place, bf16)
                    nc.vector.tensor_tensor(
                        out=gt[:].rearrange("p (k e) -> p k e", e=D),
                        in0=gt[:].rearrange("p (k e) -> p k e", e=D),
                        in1=wm[:].to_broadcast([P, 2 * ncols, D]),
                        op=OP.mult)
                    # pairwise tree-sum over the 2*Cc slot axis (contiguous)
                    m = 2 * Cc
                    cur = gt
                    cw = m
                    lev = 0
                    while cw > 1:
                        half = cw // 2
                        rem = cw - 2 * half
                        om = half + rem
                        ndt = bf16 if lev == 0 else f32
                        ntag = "tr0b" if lev == 0 else f"tr{1 + lev % 2}"
                        nt = tp.tile([P, g * om * D], ndt, tag=ntag)
                        cur4 = cur[:].rearrange("p (g c d) -> p g c d", g=g, d=D)
                        nt4 = nt[:].rearrange("p (g c d) -> p g c d", g=g, d=D)
                        nc.vector.tensor_tensor(
                            out=nt4[:, :, 0:half, :],
                            in0=cur4[:, :, 0:half, :],
                            in1=cur4[:, :, half:2 * half, :], op=OP.add)
                        if rem:
                            nc.vector.tensor_copy(
                                out=nt4[:, :, half:half + 1, :],
                                in_=cur4[:, :, 2 * half:2 * half + 1, :])
                        cur = nt
                        cw = om
                        lev += 1
                    # U = tree + self, normalized
                    selfr = radj[:].rearrange("p (j e) -> p j e", e=D + 1)[
                        :, j0:j0 + g, 0:D]
                    U = wp.tile([P, g * D], f32, tag="U")
                    nc.vector.tensor_tensor(
                        out=U[:].rearrange("p (g d) -> p g d", d=D),
                        in0=selfr,
                        in1=wself[:, j0:j0 + g].to_broadcast([P, g, D]),
                        op=OP.mult)
                    nc.vector.tensor_tensor(out=U[:], in0=U[:], in1=cur[:],
                                            op=OP.add)
                    nc.vector.tensor_tensor(
                        out=U[:].rearrange("p (g d) -> p g d", d=D),
                        in0=U[:].rearrange("p (g d) -> p g d", d=D),
                        in1=recip[:].to_broadcast([P, g, D]),
                        op=OP.mult)
                    # transpose blocks into one PSUM tile -> aggT [64, g*128]
                    at_ps = psB.tile([D, g * P], f32, tag="at")
                    for gg in range(g):
                        nc.tensor.transpose(
                            out=at_ps[:, gg * P:(gg + 1) * P],
                            in_=U[:, gg * D:(gg + 1) * D],
                            identity=ident[:])
                    aggT = wp.tile([D, g * P], f32, tag="aggT")
                    nc.vector.tensor_copy(out=aggT[:], in_=at_ps[:])
                    if l < N_LAYERS - 1:
                        h_ps = psB.tile([D, g * P], f32, tag="hps")
                        nc.tensor.matmul(
                            out=h_ps[:], lhsT=Qst[:, l * D:(l + 1) * D],
                            rhs=aggT[:], start=True, stop=True)
                        nc.vector.tensor_scalar(
                            out=hT[:, j0 * P:(j0 + g) * P], in0=h_ps[:],
                            scalar1=bcol[:, l:l + 1], scalar2=0.0,
                            op0=OP.add, op1=OP.max)
                    else:
                        for gg in range(g):
                            h3_ps = psB.tile([P, D], f32, tag="h3ps")
                            nc.tensor.matmul(
                                out=h3_ps[:],
                                lhsT=aggT[:, gg * P:(gg + 1) * P],
                                rhs=Qst[:, l * D:(l + 1) * D],
                                start=True, stop=True)
                            h3 = wp.tile([P, D], f32, tag="h3")
                            nc.vector.tensor_tensor(
                                out=h3[:], in0=h3_ps[:], in1=brow3[:], op=OP.add)
                            nc.sync.dma_start(
                                t_h3loc[(j0 + gg) * P:(j0 + gg + 1) * P, :], h3[:])
            # ---- pooling: segment max over graphs ----
            for q in range(GB):
                PCq = int(PC[q])
                pg_t = gp.tile([P, PCq * D], f32, tag="pg")
                pg_g = pg_t[:].rearrange("p (k e) -> p k e", e=D)
                a = 0
                while a < PCq:
                    b = min(a + MAX_GATHER_COLS, PCq)
                    nidx = (b - a) * P
                    nc.gpsimd.dma_gather(
                        out_ap=pg_g[:, a:b, :],
                        in_ap=t_h3loc[:],
                        idxs_ap=pidx[:, 8 * (poffs[q] + a):8 * (poffs[q] + b)],
                        num_idxs=nidx,
                        num_idxs_reg=nidx,
                        elem_size=D,
                        single_packet=False,
                    )
                    a = b
                cur = pg_t
                cw = PCq
                lev = 0
                while cw > 1:
                    half = cw // 2
                    rem = cw - 2 * half
                    om = half + rem
                    nt = tp.tile([P, om * D], f32, tag=f"pt{lev % 2}")
                    cur3 = cur[:].rearrange("p (c d) -> p c d", d=D)
                    nt3 = nt[:].rearrange("p (c d) -> p c d", d=D)
                    nc.vector.tensor_tensor(
                        out=nt3[:, 0:half, :], in0=cur3[:, 0:half, :],
                        in1=cur3[:, half:2 * half, :], op=OP.max)
                    if rem:
                        nc.vector.tensor_copy(
                            out=nt3[:, half:half + 1, :],
                            in_=cur3[:, 2 * half:2 * half + 1, :])
                    cur = nt
                    cw = om
                    lev += 1
                nc.sync.dma_start(t_gpart[q * P:(q + 1) * P, :], cur[:, 0:D])
            nc.gpsimd.collective_compute(
                "AllReduce", mybir.AluOpType.max,
                replica_groups=[list(range(N_CORES))],
                ins=[t_gpart[:].opt()],
                outs=[t_gall[:].opt()],
            )
            # ---- fc + log_softmax (redundant on all cores) ----
            for q in range(GB):
                gsb = wp.tile([P, D], f32, tag="gsb")
                nc.sync.dma_start(gsb[:], t_gall[q * P:(q + 1) * P, :])
                mask = wp.tile([P, D], f32, tag="mask")
                nc.vector.tensor_scalar(
                    out=mask[:], in0=gsb[:], scalar1=-1.0e37, scalar2=None,
                    op0=OP.is_gt)
                nc.vector.tensor_tensor(out=gsb[:], in0=gsb[:], in1=mask[:],
                                        op=OP.mult)
                gT_ps = psB.tile([D, P], f32, tag="gT")
                nc.tensor.transpose(out=gT_ps[:], in_=gsb[:], identity=ident[:])
                gT = wp.tile([D, P], f32, tag="gTs")
                nc.vector.tensor_copy(out=gT[:], in_=gT_ps[:])
                lg_ps = psB.tile([P, N_CLASSES], f32, tag="lg")
                nc.tensor.matmul(out=lg_ps[:], lhsT=gT[:], rhs=fcwT[:],
                                 start=True, stop=True)
                lg = wp.tile([P, N_CLASSES], f32, tag="lgs")
                nc.vector.tensor_tensor(
                    out=lg[:], in0=lg_ps[:], in1=fcb[:], op=OP.add)
                m = wp.tile([P, 1], f32, tag="m")
                nc.vector.reduce_max(out=m[:], in_=lg[:], axis=mybir.AxisListType.X)
                mneg = wp.tile([P, 1], f32, tag="mneg")
                nc.vector.tensor_scalar_mul(out=mneg[:], in0=m[:], scalar1=-1.0)
                ex = wp.tile([P, N_CLASSES], f32, tag="ex")
                sumex = wp.tile([P, 1], f32, tag="sumex")
                nc.scalar.activation(out=ex[:], in_=lg[:], func=AF.Exp,
                                     bias=mneg[:], accum_out=sumex[:])
                logz = wp.tile([P, 1], f32, tag="logz")
                nc.scalar.activation(out=logz[:], in_=sumex[:], func=AF.Ln)
                off = wp.tile([P, 1], f32, tag="off")
                nc.vector.tensor_add(out=off[:], in0=m[:], in1=logz[:])
                outsb = wp.tile([P, N_CLASSES], f32, tag="outsb")
                nc.vector.tensor_tensor(
                    out=outsb[:], in0=lg[:],
                    in1=off[:].to_broadcast([P, N_CLASSES]), op=OP.subtract)
                nc.sync.dma_start(t_out[q * P:(q + 1) * P, :], outsb[:])
    nc.compile()
    return nc


def kernel(**inputs):
    x = np.asarray(inputs["x"])
    key = (x.shape, inputs["edge_index"].shape)
    host, per_core = _host_prep(**inputs)
    if key not in _COMPILED:
        _COMPILED[key] = _build(host)
    nc = _COMPILED[key]
    in_maps = [per_core[c] for c in range(N_CORES)]
    import os
    trace = False
    if os.environ.get("KERNEL_TRACE") == "1":
        try:
            import types
            if "antenv.axon_hooks" not in sys.modules:
                import antenv
                from trn_agent_boot.trn_boot import _ntff_profile_via_ctypes
                mod = types.ModuleType("antenv.axon_hooks")
                _state = {"hook": _ntff_profile_via_ctypes("/opt/axon/libaxon_pjrt.so")}
                mod.set_axon_ntff_profile_hook = lambda h: _state.__setitem__("hook", h)
                mod.get_axon_ntff_profile_hook = lambda: _state["hook"]
                sys.modules["antenv.axon_hooks"] = mod
                antenv.axon_hooks = mod
            trace = True
        except Exception:
            trace = False
    res = bass_utils.run_bass_kernel_spmd(
        nc, in_maps, core_ids=list(range(N_CORES)), trace=trace)
    globals()['LAST_EXEC_NS'] = res.exec_time_ns
    return np.asarray(res.results[0]["out"], np.float32)


LAST_EXEC_NS = None


# revision 8
# speedup vs baseline: 1.3337x; 1.2520x over previous
"""Distributed GAT (3-layer, heads=1) Bass kernel for 8 TRN2 NeuronCores.

Strategy (dst-sharded, batched dma_gather over a bf16 pair-row table):
- Host: permute nodes by in-degree (excl. self-loop) into degree-homogeneous
  blocks of 128, deal blocks round-robin to 8 cores. Table row r = node;
  pair-row i = nodes (2i, 2i+1) packed as 128 bf16 = 256 B, so pair indices
  fit int16 (max 25087 < 32767) and one InstDMAGatherAnt fetches thousands
  of rows per instruction (vs one 128-row indirect DMA per slot column).
- Blocks are grouped into chunks of G=4; per-chunk slot capacity Cc = max
  in-degree in the chunk's rank groups. Slots gather the PAIR containing the
  src node; a static half-mask kills the wrong half and pad slots.
- Device per layer:
    node pass:  per block one matmul r=[h@(W Q) | h@(W a_dst)] -> radj (bf16)
                kept in SBUF (self-loop contributions read locally) and
                DMA'd row-major to tabA.
    exchange:   AllGather (bf16) -> Shared tabFull [NPAD, 64].
    edge pass:  per chunk: dma_gather pair rows -> [128, G*Cc, 128] bf16;
                w = max(exp(e), exp(0.2 e)) (Exp-only scalar table; no Lrelu
                table thrash), masked; unnormalized weighted sums via
                contiguous pairwise tree reduction (no strided reduce);
                add self term, normalize by the accumulated denominator,
                transpose+reconstruct through Q per 4-block PE group with
                fused bias+relu on DVE.
- Pooling: segment-max via dma_gather from local h3loc (sentinel -3e38),
  pairwise max tree, AllReduce(max), then fc + log_softmax on every core.
"""
import sys

sys.path.insert(0, "/opt/trn_rl_repo")

import numpy as np

import concourse.bass as bass
import concourse.bacc as bacc
import concourse.tile as tile
import concourse.mybir as mybir
from concourse import bass_utils
from concourse.masks import make_identity

N_CORES = 8
D = 64
N_LAYERS = 3
N_CLASSES = 10
N_GRAPHS = 512
NEG_SLOPE = 0.2
P = 128
GBLK = 4                  # blocks per chunk
MAX_GATHER_COLS = 118     # 128*cols idx ints must fit Q7 scratch (<=~16200)

_COMPILED = {}


def _householder_first_col(a):
    """Orthogonal symmetric Q with Q @ e0 = a/||a||."""
    a = np.asarray(a, np.float64)
    ah = a / np.linalg.norm(a)
    e0 = np.zeros_like(ah)
    e0[0] = 1.0
    u = ah - e0
    nu = np.linalg.norm(u)
    if nu < 1e-12:
        return np.eye(len(a))
    u = u / nu
    return np.eye(len(a)) - 2.0 * np.outer(u, u)


def _idx16_of(V):
    """[128, S] int -> int16 idx tile [128, 8S] (16-part pattern replicated x8).

    Gather position i = col*128 + p reads idx[i%16, i//16]; with
    i//16 = col*8 + p//16 the transform is a reshape/transpose.
    """
    Pn, S = V.shape
    assert Pn == 128
    t = V.reshape(8, 16, S).transpose(1, 2, 0).reshape(16, 8 * S)
    return np.tile(t.astype(np.int16), (8, 1))


def _host_prep(x, edge_index, batch, Ws, a_src, a_dst, bs, fc_w, fc_b):
    N = x.shape[0]
    src0 = np.asarray(edge_index[0], np.int64)
    dst0 = np.asarray(edge_index[1], np.int64)
    batch = np.asarray(batch, np.int64)

    NBLK_TOT = -(-N // P)
    NBLK_TOT = ((NBLK_TOT + N_CORES - 1) // N_CORES) * N_CORES
    NPAD = NBLK_TOT * P
    NB = NBLK_TOT // N_CORES
    NPC = NB * P
    NPAIR = NPAD // 2

    # in-degree (excluding self loops; those are handled locally on-chip)
    deg = np.zeros(NPAD, np.int64)
    np.add.at(deg, dst0, 1)
    order = np.argsort(-deg, kind="stable")
    new_id = np.empty(NPAD, np.int64)
    new_id[order] = np.arange(NPAD)
    k = np.arange(NPAD) // P
    p_in_blk = np.arange(NPAD) % P
    row_of_pos = (k % N_CORES) * NPC + (k // N_CORES) * P + p_in_blk
    row_of_old = row_of_pos[new_id]

    src_r = row_of_old[src0]
    dst_r = row_of_old[dst0]

    # per (core, block, partition) in-edge counts -> rank-group capacities
    core_of = dst_r // NPC
    j_of = (dst_r % NPC) // P
    p_of = dst_r % P
    cnt = np.zeros((N_CORES, NB, P), np.int64)
    np.add.at(cnt, (core_of, j_of, p_of), 1)
    C = cnt.max(axis=(0, 2))  # [NB] shared across cores

    # chunks of up to GBLK consecutive blocks, padded to chunk max
    chunks = []  # (j0, g, Cc, ocol)
    ocol = 0
    j0 = 0
    while j0 < NB:
        g = min(GBLK, NB - j0)
        Cc = int(C[j0:j0 + g].max())
        Cc = max(Cc, 1)
        chunks.append((j0, g, Cc, ocol))
        ocol += g * Cc
        j0 += g
    S = ocol

    # slot fill (vectorized): rank of each edge within its dst
    eorder = np.argsort(dst_r, kind="stable")
    dst_s = dst_r[eorder]
    src_s = src_r[eorder]
    dcount = np.zeros(NPAD + 1, np.int64)
    np.add.at(dcount, dst_s, 1)
    starts = np.zeros(NPAD + 1, np.int64)
    np.cumsum(dcount[:-1], out=starts[1:])
    rank_e = np.arange(len(dst_s)) - starts[dst_s]

    ci_of_j = np.zeros(NB, np.int64)
    colbase_of_j = np.zeros(NB, np.int64)
    Cc_of_j = np.zeros(NB, np.int64)
    for ci, (j0, g, Cc, oc) in enumerate(chunks):
        for gg in range(g):
            ci_of_j[j0 + gg] = ci
            colbase_of_j[j0 + gg] = oc + gg * Cc
            Cc_of_j[j0 + gg] = Cc

    ecore = dst_s // NPC
    ej = (dst_s % NPC) // P
    ep = dst_s % P
    ecol = colbase_of_j[ej] + rank_e

    slotpair = np.zeros((N_CORES, P, S), np.int64)
    halfmask = np.zeros((N_CORES, P, 2 * S), np.float32)
    slotpair[ecore, ep, ecol] = src_s // 2
    halfmask[ecore, ep, 2 * ecol + (src_s % 2)] = 1.0

    # pooling: local rows grouped by graph (graph block q = graphs 128q..)
    GB = N_GRAPHS // P
    graph_of_row = np.full(NPAD, -1, np.int64)
    graph_of_row[row_of_old[:N]] = batch
    pool_cnt = np.zeros((N_CORES, GB, P), np.int64)
    rr = np.arange(NPAD)
    real = graph_of_row >= 0
    prc = rr[real] // NPC
    pg = graph_of_row[real]
    np.add.at(pool_cnt, (prc, pg // P, pg % P), 1)
    PC = np.maximum(pool_cnt.max(axis=(0, 2)), 1)  # [GB]
    poffs = np.zeros(GB + 1, np.int64)
    np.cumsum(PC, out=poffs[1:])
    SG = int(poffs[-1])
    LSENT = NPC
    pool_slot = np.full((N_CORES, P, SG), LSENT, np.int64)
    # rank of local row within its (core, graph)
    lr = rr[real]
    lcore = prc
    lloc = lr % NPC
    pkey = lcore * N_GRAPHS + pg
    porder = np.argsort(pkey, kind="stable")
    pk_s = pkey[porder]
    lloc_s = lloc[porder]
    pstart = np.zeros(N_CORES * N_GRAPHS + 1, np.int64)
    pc2 = np.zeros(N_CORES * N_GRAPHS + 1, np.int64)
    np.add.at(pc2, pk_s, 1)
    np.cumsum(pc2[:-1], out=pstart[1:])
    prank = np.arange(len(pk_s)) - pstart[pk_s]
    pcore_s = pk_s // N_GRAPHS
    pgr_s = pk_s % N_GRAPHS
    pool_slot[pcore_s, pgr_s % P, poffs[pgr_s // P] + prank] = lloc_s

    # weights
    Ws = np.asarray(Ws, np.float64)
    a_src = np.asarray(a_src, np.float64)
    a_dst = np.asarray(a_dst, np.float64)
    bs = np.asarray(bs, np.float64)
    NR = np.zeros((N_LAYERS, D, D + 1), np.float64)
    Qs = np.zeros((N_LAYERS, D, D), np.float64)
    s_l = np.zeros(N_LAYERS)
    for l in range(N_LAYERS):
        Q = _householder_first_col(a_src[l])
        Qs[l] = Q
        s_l[l] = np.linalg.norm(a_src[l])
        NR[l, :, :D] = Ws[l] @ Q
        NR[l, :, D] = Ws[l] @ a_dst[l]

    xpad = np.zeros((NPAD, D), np.float32)
    xpad[row_of_old[:N]] = np.asarray(x, np.float32)

    import ml_dtypes
    bf = ml_dtypes.bfloat16

    host = dict(
        NPAD=NPAD, NB=NB, NPC=NPC, NPAIR=NPAIR, S=S, SG=SG, GB=GB,
        chunks=chunks, PC=PC.astype(int), poffs=poffs.astype(int),
        s_l=s_l,
    )
    per_core = []
    for c in range(N_CORES):
        per_core.append({
            "xT": np.ascontiguousarray(
                xpad[c * NPC:(c + 1) * NPC].T.astype(bf)),
            "idx": np.ascontiguousarray(_idx16_of(slotpair[c])),
            "pidx": np.ascontiguousarray(_idx16_of(pool_slot[c])),
            "hmask": np.ascontiguousarray(halfmask[c].astype(bf)),
            "NR": np.ascontiguousarray(
                NR.transpose(1, 0, 2).reshape(D, N_LAYERS * (D + 1)).astype(bf)),
            "Qs": np.ascontiguousarray(
                Qs.transpose(1, 0, 2).reshape(D, N_LAYERS * D).astype(np.float32)),
            "bcol": np.ascontiguousarray(bs.T.astype(np.float32)),
            "brow3": np.ascontiguousarray(
                np.tile(bs[2][None, :].astype(np.float32), (P, 1))),
            "fcwT": np.ascontiguousarray(np.asarray(fc_w, np.float32).T),
            "fcb": np.ascontiguousarray(
                np.tile(np.asarray(fc_b, np.float32)[None, :], (P, 1))),
        })
    return host, per_core


def _build(host):
    NB, NPC, S, SG, GB = host["NB"], host["NPC"], host["S"], host["SG"], host["GB"]
    NPAD, NPAIR = host["NPAD"], host["NPAIR"]
    chunks = host["chunks"]
    PC, poffs = host["PC"], host["poffs"]
    s_l = host["s_l"]
    f32 = mybir.dt.float32
    bf16 = mybir.dt.bfloat16
    i16 = mybir.dt.int16
    AF = mybir.ActivationFunctionType
    OP = mybir.AluOpType

    nc = bacc.Bacc("TRN2", target_bir_lowering=False, debug=False, num_devices=N_CORES,
                   num_swdge_queues=4)
    t_xT = nc.dram_tensor("xT", [D, NPC], bf16, kind="ExternalInput")
    t_idx = nc.dram_tensor("idx", [P, 8 * S], i16, kind="ExternalInput")
    t_pidx = nc.dram_tensor("pidx", [P, 8 * SG], i16, kind="ExternalInput")
    t_hmask = nc.dram_tensor("hmask", [P, 2 * S], bf16, kind="ExternalInput")
    t_NR = nc.dram_tensor("NR", [D, N_LAYERS * (D + 1)], bf16, kind="ExternalInput")
    t_Qs = nc.dram_tensor("Qs", [D, N_LAYERS * D], f32, kind="ExternalInput")
    t_bcol = nc.dram_tensor("bcol", [D, N_LAYERS], f32, kind="ExternalInput")
    t_brow3 = nc.dram_tensor("brow3", [P, D], f32, kind="ExternalInput")
    t_fcwT = nc.dram_tensor("fcwT", [D, N_CLASSES], f32, kind="ExternalInput")
    t_fcb = nc.dram_tensor("fcb", [P, N_CLASSES], f32, kind="ExternalInput")
    t_out = nc.dram_tensor("out", [N_GRAPHS, N_CLASSES], f32, kind="ExternalOutput")

    t_tabA = nc.dram_tensor("tabA", [NPC, D], bf16)
    t_tabFull = nc.dram_tensor("tabFull", [NPAD, D], bf16, addr_space="Shared")
    t_h3loc = nc.dram_tensor("h3loc", [NPC + 1, D], f32)
    t_gpart = nc.dram_tensor("gpart", [N_GRAPHS, D], f32)
    t_gall = nc.dram_tensor("gall", [N_GRAPHS, D], f32)

    with tile.TileContext(nc) as tc:
        with (
            tc.tile_pool(name="persist", bufs=1) as pp,
            tc.tile_pool(name="gt", bufs=2) as gp,
            tc.tile_pool(name="work", bufs=2) as wp,
            tc.tile_pool(name="psA", bufs=2, space="PSUM") as psA,
            tc.tile_pool(name="psB", bufs=1, space="PSUM") as psB,
        ):
            # persistent SBUF
            hT = pp.tile([D, NPC], bf16)
            radj = pp.tile([P, NB * (D + 1)], bf16)
            idxs = pp.tile([P, 8 * S], i16)
            pidx = pp.tile([P, 8 * SG], i16)
            hmask = pp.tile([P, 2 * S], bf16)
            NRt = pp.tile([D, N_LAYERS * (D + 1)], bf16)
            Qst = pp.tile([D, N_LAYERS * D], f32)
            bcol = pp.tile([D, N_LAYERS], f32)
            brow3 = pp.tile([P, D], f32)
            fcwT = pp.tile([D, N_CLASSES], f32)
            fcb = pp.tile([P, N_CLASSES], f32)
            ident = pp.tile([P, P], f32)
            sentP = pp.tile([1, D], f32)
            nc.sync.dma_start(hT[:], t_xT[:])
            nc.sync.dma_start(idxs[:], t_idx[:])
            nc.sync.dma_start(pidx[:], t_pidx[:])
            nc.sync.dma_start(hmask[:], t_hmask[:])
            nc.sync.dma_start(NRt[:], t_NR[:])
            nc.sync.dma_start(Qst[:], t_Qs[:])
            nc.sync.dma_start(bcol[:], t_bcol[:])
            nc.sync.dma_start(brow3[:], t_brow3[:])
            nc.sync.dma_start(fcwT[:], t_fcwT[:])
            nc.sync.dma_start(fcb[:], t_fcb[:])
            make_identity(nc, ident[:])
            nc.vector.memset(sentP[:], -3.0e38)
            nc.sync.dma_start(t_h3loc[NPC:NPC + 1, :], sentP[:])

            tab_pairs = t_tabFull[:].rearrange("(a b) d -> a (b d)", b=2)
            gq = [0]

            for l in range(N_LAYERS):
                sl = float(s_l[l])
                # ---- node pass ----
                for j in range(NB):
                    np_ps = psA.tile([P, D + 1], f32, tag="npp")
                    nc.tensor.matmul(
                        out=np_ps[:],
                        lhsT=hT[:, j * P:(j + 1) * P],
                        rhs=NRt[:, l * (D + 1):(l + 1) * (D + 1)],
                        start=True, stop=True,
                    )
                    nc.scalar.activation(
                        out=radj[:, j * (D + 1):(j + 1) * (D + 1)], in_=np_ps[:],
                        func=AF.Copy)
                    nc.sync.dma_start(
                        t_tabA[j * P:(j + 1) * P, :],
                        radj[:, j * (D + 1):j * (D + 1) + D])
                # ---- exchange ----
                nc.gpsimd.collective_compute(
                    "AllGather", mybir.AluOpType.bypass,
                    replica_groups=[list(range(N_CORES))],
                    ins=[t_tabA[:].opt()],
                    outs=[t_tabFull[:].opt()],
                )
                # ---- self-loop weights for all blocks: w = max(e^e2, e^.2e2)
                r0s = radj[:].rearrange("p (j e) -> p j e", e=D + 1)[:, :, 0:1] \
                    .rearrange("p j one -> p (j one)")
                ads = radj[:].rearrange("p (j e) -> p j e", e=D + 1)[:, :, D:D + 1] \
                    .rearrange("p j one -> p (j one)")
                adc = wp.tile([P, NB], f32, tag="adc")
                nc.scalar.activation(out=adc[:], in_=ads, func=AF.Copy)
                e2s = wp.tile([P, NB], f32, tag="e2s")
                nc.scalar.activation(out=e2s[:], in_=r0s, func=AF.Copy, scale=sl)
                nc.vector.tensor_tensor(out=e2s[:], in0=e2s[:], in1=adc[:], op=OP.add)
                wsA = wp.tile([P, NB], f32, tag="wsA")
                wself = wp.tile([P, NB], f32, tag="wself")
                nc.scalar.activation(out=wsA[:], in_=e2s[:], func=AF.Exp)
                nc.scalar.activation(out=wself[:], in_=e2s[:], func=AF.Exp,
                                     scale=NEG_SLOPE)
                nc.vector.tensor_tensor(out=wself[:], in0=wself[:], in1=wsA[:],
                                        op=OP.max)
                # ---- edge pass per chunk ----
                for (j0, g, Cc, oc) in chunks:
                    ncols = g * Cc
                    gt = gp.tile([P, ncols * P], bf16, tag="gt")
                    gt_g = gt[:].rearrange("p (k e) -> p k e", e=P)
                    npieces = -(-ncols // MAX_GATHER_COLS)
                    a = 0
                    for pi in range(npieces):
                        b = a + (ncols - a) // (npieces - pi)
                        nidx = (b - a) * P
                        nc.gpsimd.dma_gather(
                            out_ap=gt_g[:, a:b, :],
                            in_ap=tab_pairs,
                            idxs_ap=idxs[:, 8 * (oc + a):8 * (oc + b)],
                            num_idxs=nidx,
                            num_idxs_reg=nidx,
                            elem_size=P,
                            single_packet=False,
                            queue_num=gq[0] % 4,
                        )
                        gq[0] += 1
                        a = b
                    # attention weights over [P, 2*ncols]
                    r0 = gt[:].rearrange("p (k e) -> p k e", e=D)[:, :, 0:1] \
                        .rearrange("p k one -> p (k one)")
                    e2 = wp.tile([P, 2 * ncols], f32, tag="e2")
                    nc.scalar.activation(out=e2[:], in_=r0, func=AF.Copy, scale=sl)
                    nc.vector.tensor_tensor(
                        out=e2[:].rearrange("p (g c) -> p g c", g=g),
                        in0=e2[:].rearrange("p (g c) -> p g c", g=g),
                        in1=adc[:, j0:j0 + g].to_broadcast([P, g, 2 * Cc]),
                        op=OP.add)
                    wA = wp.tile([P, 2 * ncols], f32, tag="wA")
                    wm = wp.tile([P, 2 * ncols], f32, tag="wm")
                    nc.scalar.activation(out=wA[:], in_=e2[:], func=AF.Exp)
                    nc.scalar.activation(out=wm[:], in_=e2[:], func=AF.Exp,
                                         scale=NEG_SLOPE)
                    nc.vector.tensor_tensor(out=wm[:], in0=wm[:], in1=wA[:],
                                            op=OP.max)
                    nc.vector.tensor_tensor(
                        out=wm[:], in0=wm[:],
                        in1=hmask[:, 2 * oc:2 * (oc + ncols)], op=OP.mult)
                    # denominator per block
                    dn = wp.tile([P, g], f32, tag="dn")
                    nc.vector.reduce_sum(
                        out=dn[:],
                        in_=wm[:].rearrange("p (g c) -> p g c", g=g),
                        axis=mybir.AxisListType.X)
                    nc.vector.tensor_tensor(
                        out=dn[:], in0=dn[:], in1=wself[:, j0:j0 + g], op=OP.add)
                    recip = wp.tile([P, g], f32, tag="recip")
                    nc.vector.reciprocal(out=recip[:], in_=dn[:])
                    # weight the gathered rows (in place, pure bf16)
                    wmb = wp.tile([P, 2 * ncols], bf16, tag="wmb")
                    nc.vector.tensor_copy(out=wmb[:], in_=wm[:])
                    nc.vector.tensor_tensor(
                        out=gt[:].rearrange("p (k e) -> p k e", e=D),
                        in0=gt[:].rearrange("p (k e) -> p k e", e=D),
                        in1=wmb[:].to_broadcast([P, 2 * ncols, D]),
                        op=OP.mult)
                    # in-place pairwise tree-sum over the 2*Cc slot axis
                    gt4 = gt[:].rearrange("p (g c d) -> p g c d", g=g, d=D)
                    cw = 2 * Cc
                    while cw > 1:
                        half = cw // 2
                        rem = cw - 2 * half
                        nc.vector.tensor_tensor(
                            out=gt4[:, :, 0:half, :],
                            in0=gt4[:, :, 0:half, :],
                            in1=gt4[:, :, half + rem:cw, :], op=OP.add)
                        cw = half + rem
                    # U = tree + self, normalized
                    selfr = radj[:].rearrange("p (j e) -> p j e", e=D + 1)[
                        :, j0:j0 + g, 0:D]
                    U = wp.tile([P, g * D], f32, tag="U")
                    nc.vector.tensor_tensor(
                        out=U[:].rearrange("p (g d) -> p g d", d=D),
                        in0=selfr,
                        in1=wself[:, j0:j0 + g].to_broadcast([P, g, D]),
                        op=OP.mult)
                    nc.vector.tensor_tensor(
                        out=U[:].rearrange("p (g d) -> p g d", d=D),
                        in0=U[:].rearrange("p (g d) -> p g d", d=D),
                        in1=gt4[:, :, 0:1, :].rearrange("p g one d -> p (g one) d"),
                        op=OP.add)
                    nc.vector.tensor_tensor(
                        out=U[:].rearrange("p (g d) -> p g d", d=D),
                        in0=U[:].rearrange("p (g d) -> p g d", d=D),
                        in1=recip[:].to_broadcast([P, g, D]),
                        op=OP.mult)
                    # transpose blocks into one PSUM tile -> aggT [64, g*128]
                    at_ps = psB.tile([D, g * P], f32, tag="at")
                    for gg in range(g):
                        nc.tensor.transpose(
                            out=at_ps[:, gg * P:(gg + 1) * P],
                            in_=U[:, gg * D:(gg + 1) * D],
                            identity=ident[:])
                    aggT = wp.tile([D, g * P], f32, tag="aggT")
                    nc.scalar.activation(out=aggT[:], in_=at_ps[:], func=AF.Copy)
                    if l < N_LAYERS - 1:
                        h_ps = psB.tile([D, g * P], f32, tag="hps")
                        nc.tensor.matmul(
                            out=h_ps[:], lhsT=Qst[:, l * D:(l + 1) * D],
                            rhs=aggT[:], start=True, stop=True)
                        nc.vector.tensor_scalar(
                            out=hT[:, j0 * P:(j0 + g) * P], in0=h_ps[:],
                            scalar1=bcol[:, l:l + 1], scalar2=0.0,
                            op0=OP.add, op1=OP.max)
                    else:
                        for gg in range(g):
                            h3_ps = psB.tile([P, D], f32, tag="h3ps")
                            nc.tensor.matmul(
                                out=h3_ps[:],
                                lhsT=aggT[:, gg * P:(gg + 1) * P],
                                rhs=Qst[:, l * D:(l + 1) * D],
                                start=True, stop=True)
                            h3 = wp.tile([P, D], f32, tag="h3")
                            nc.vector.tensor_tensor(
                                out=h3[:], in0=h3_ps[:], in1=brow3[:], op=OP.add)
                            nc.sync.dma_start(
                                t_h3loc[(j0 + gg) * P:(j0 + gg + 1) * P, :], h3[:])
            # ---- pooling: segment max over graphs ----
            for q in range(GB):
                PCq = int(PC[q])
                pg_t = gp.tile([P, PCq * D], f32, tag="pg")
                pg_g = pg_t[:].rearrange("p (k e) -> p k e", e=D)
                a = 0
                while a < PCq:
                    b = min(a + MAX_GATHER_COLS, PCq)
                    nidx = (b - a) * P
                    nc.gpsimd.dma_gather(
                        out_ap=pg_g[:, a:b, :],
                        in_ap=t_h3loc[:],
                        idxs_ap=pidx[:, 8 * (poffs[q] + a):8 * (poffs[q] + b)],
                        num_idxs=nidx,
                        num_idxs_reg=nidx,
                        elem_size=D,
                        single_packet=False,
                        queue_num=gq[0] % 4,
                    )
                    gq[0] += 1
                    a = b
                cw = PCq
                while cw > 1:
                    half = cw // 2
                    rem = cw - 2 * half
                    nc.vector.tensor_tensor(
                        out=pg_g[:, 0:half, :], in0=pg_g[:, 0:half, :],
                        in1=pg_g[:, half + rem:cw, :], op=OP.max)
                    cw = half + rem
                nc.sync.dma_start(t_gpart[q * P:(q + 1) * P, :], pg_t[:, 0:D])
            nc.gpsimd.collective_compute(
                "AllReduce", mybir.AluOpType.max,
                replica_groups=[list(range(N_CORES))],
                ins=[t_gpart[:].opt()],
                outs=[t_gall[:].opt()],
            )
            # ---- fc + log_softmax (redundant on all cores) ----
            for q in range(GB):
                gsb = wp.tile([P, D], f32, tag="gsb")
                nc.sync.dma_start(gsb[:], t_gall[q * P:(q + 1) * P, :])
                mask = wp.tile([P, D], f32, tag="mask")
                nc.vector.tensor_scalar(
                    out=mask[:], in0=gsb[:], scalar1=-1.0e37, scalar2=None,
                    op0=OP.is_gt)
                nc.vector.tensor_tensor(out=gsb[:], in0=gsb[:], in1=mask[:],
                                        op=OP.mult)
                gT_ps = psB.tile([D, P], f32, tag="gT")
                nc.tensor.transpose(out=gT_ps[:], in_=gsb[:], identity=ident[:])
                gT = wp.tile([D, P], f32, tag="gTs")
                nc.vector.tensor_copy(out=gT[:], in_=gT_ps[:])
                lg_ps = psB.tile([P, N_CLASSES], f32, tag="lg")
                nc.tensor.matmul(out=lg_ps[:], lhsT=gT[:], rhs=fcwT[:],
                                 start=True, stop=True)
                lg = wp.tile([P, N_CLASSES], f32, tag="lgs")
                nc.vector.tensor_tensor(
                    out=lg[:], in0=lg_ps[:], in1=fcb[:], op=OP.add)
                m = wp.tile([P, 1], f32, tag="m")
                nc.vector.reduce_max(out=m[:], in_=lg[:], axis=mybir.AxisListType.X)
                mneg = wp.tile([P, 1], f32, tag="mneg")
                nc.vector.tensor_scalar_mul(out=mneg[:], in0=m[:], scalar1=-1.0)
                ex = wp.tile([P, N_CLASSES], f32, tag="ex")
                sumex = wp.tile([P, 1], f32, tag="sumex")
                nc.scalar.activation(out=ex[:], in_=lg[:], func=AF.Exp,
                                     bias=mneg[:], accum_out=sumex[:])
                logz = wp.tile([P, 1], f32, tag="logz")
                nc.scalar.activation(out=logz[:], in_=sumex[:], func=AF.Ln)
                off = wp.tile([P, 1], f32, tag="off")
                nc.vector.tensor_add(out=off[:], in0=m[:], in1=logz[:])
                outsb = wp.tile([P, N_CLASSES], f32, tag="outsb")
                nc.vector.tensor_tensor(
                    out=outsb[:], in0=lg[:],
                    in1=off[:].to_broadcast([P, N_CLASSES]), op=OP.subtract)
                nc.sync.dma_start(t_out[q * P:(q + 1) * P, :], outsb[:])
    nc.compile()
    return nc


def kernel(**inputs):
    x = np.asarray(inputs["x"])
    key = (x.shape, inputs["edge_index"].shape)
    host, per_core = _host_prep(**inputs)
    if key not in _COMPILED:
        _COMPILED[key] = _build(host)
    nc = _COMPILED[key]
    in_maps = [per_core[c] for c in range(N_CORES)]
    import os
    trace = False
    if os.environ.get("KERNEL_TRACE") == "1":
        try:
            import types
            if "antenv.axon_hooks" not in sys.modules:
                import antenv
                from trn_agent_boot.trn_boot import _ntff_profile_via_ctypes
                mod = types.ModuleType("antenv.axon_hooks")
                _state = {"hook": _ntff_profile_via_ctypes("/opt/axon/libaxon_pjrt.so")}
                mod.set_axon_ntff_profile_hook = lambda h: _state.__setitem__("hook", h)
                mod.get_axon_ntff_profile_hook = lambda: _state["hook"]
                sys.modules["antenv.axon_hooks"] = mod
                antenv.axon_hooks = mod
            trace = True
        except Exception:
            trace = False
    res = bass_utils.run_bass_kernel_spmd(
        nc, in_maps, core_ids=list(range(N_CORES)), trace=trace)
    globals()['LAST_EXEC_NS'] = res.exec_time_ns
    return np.asarray(res.results[0]["out"], np.float32)


LAST_EXEC_NS = None
